# revision 21
# baseline (speedup 1.0000x reference)
"""Trainium2 Bass kernel for nn_Encoder_60318520705555 (DGCNN-style encoder).

Sharding: data-parallel over batch B=8 across 8 NeuronCores (1 batch element
per core); BN batch statistics are all-reduced across cores (6 tiny
AllReduces). Everything else is core-local.

Self-contained: hardcodes shapes (B=8, N=2048, K=16, channel sizes).

Perf/accuracy design:
  - phase 1 (xyz knn -> covariance features) is GATHER-FREE: after the top-16
    VALUES (max8 / match_replace8 / max8 on DVE), a 0/1 fp16 selection mask
    M[p,j] = (ut[p,j] >= 16th value) is built in one DVE pass; neighbor sums
    [S(3), SS(9)] come from PE matmuls (M^T chunk) @ P12 against a hi/lo fp16
    product table, and cov = SS - S S^T/16.  This removes all 256 phase-1
    indirect DMAs (Pool) and both FIND_INDEX8 passes per block (DVE).
    (InstDMAGatherAnt and multi-index indirect DMAs both crash this runtime's
    Q7/SWDGE -- HW-tested -- so graph-layer gathers stay 16x single-index
    SWDGE indirect DMAs per block, the graph-phase cadence limit.)
  - all distance matmuls take fp16 inputs with f32 PSUM accumulation; phase-1
    uses an exact hi/lo split (11 contraction rows, error ~2^-22); the graph
    layers ride the "-|f|^2" free-axis term as fp16 hi/lo rows.  Top-k
    compare runs on f32 values (16-bit compare flips near-tied selections:
    measured 7e-2 bf16 / 9e-3 fp16 / 6e-4 f32).  DVE max8/match/find run at
    the same speed for f32 and fp16 (no 2x uop variants), so f32 is free.
  - activations, feature tables, gathers and the g1/g2/c4 conv weights are
    fp16 (value error only, ~1e-4..1e-3). Phase-1 geometry (covariances
    suffer catastrophic cancellation) and all BN statistics stay fp32.
"""

import sys

sys.path.insert(0, "/opt/trn_rl_repo")

import numpy as np

import bass_rust
import concourse.bass as bass
import concourse.mybir as mybir
import concourse.tile as tile
from concourse.bass import IndirectOffsetOnAxis
from concourse.bass_utils import run_bass_kernel_spmd
from concourse.masks import make_identity
from concourse.library_config import all_libraries, standard

F32 = mybir.dt.float32
F16 = mybir.dt.float16
U32 = mybir.dt.uint32
I16 = mybir.dt.int16
AF = mybir.ActivationFunctionType
AX = mybir.AxisListType
OP = mybir.AluOpType

# dma_gather (one Pool instruction per block) vs 16x indirect SWDGE DMAs
# (994ns fixed Pool cost each, but HW-validated)
USE_DMA_GATHER = False
P1_ROWS = 11  # phase-1 hi/lo fp16 distance matmul contraction rows

N_CORES = 8
B = 8
N = 2048
KNN = 16
NB = N // 128  # row blocks
BN_EPS = 1e-5
NEG = -1.0e30
INV_M = 1.0 / (B * N)  # BN mean divisor (global batch)

# conv layer channel sizes
C1_IN, C1_OUT = 12, 64
C2_OUT, C3_OUT = 64, 64
G1_OUT, G2_OUT = 128, 1024
C4_OUT = 512


def ts(i, s):
    return slice(i * s, (i + 1) * s)


def split_drain_waits(nc, limit=1):
    """walrus core_v3 codegen rejects instructions carrying more than one
    sync wait; hoist excess waits onto single-wait NoOp carriers just
    before the instruction (engine streams are in-order, so this is
    semantically equivalent)."""
    for f in nc.m.functions:
        for bb in f.blocks:
            out = []
            changed = False
            for inst in bb.instructions:
                si = inst.sync_info
                if si is not None and len(si.on_wait) > limit:
                    waits = list(si.on_wait)
                    chunks = [waits[i : i + limit] for i in range(0, len(waits), limit)]
                    for j, ch in enumerate(chunks[:-1]):
                        d = mybir.InstNoOp(name=f"{inst.name}-sw{j}", engine=inst.engine)
                        d.sync_info = bass_rust.SyncInfo(on_wait=ch, on_update=[])
                        nc.register_instruction(d, overwrite=True)
                        out.append(d)
                    si.on_wait = chunks[-1]
                    inst.sync_info = si
                    changed = True
                out.append(inst)
            if changed:
                bb.instructions = out


def build_program():
    nc = bass.Bass()

    # ---- I/O declarations (per-core shapes; host prepares the layouts) ----
    inp = {}

    def din(name, shape, dt=F32):
        inp[name] = nc.dram_tensor(name, list(shape), dt, kind="ExternalInput")
        return inp[name]

    # phase-1 distance matmul, exact via fp16 hi/lo split (error ~2^-22):
    # Lt1 = [2x_hi(3); 2x_hi(3); 2x_lo(3); 1; 1]
    # Rt1 = [x_hi(3);  x_lo(3);  x_hi(3); -aa_hi; -aa_lo]
    din("Lt1", (P1_ROWS, N), F16)
    din("Rt1", (P1_ROWS, N), F16)
    din("xpad", (N, 4))     # x padded to 4 cols (16B rows, self-x loads)
    din("P12", (N, 24), F16)  # [x(3), x_c*x_d(9)] hi/lo product table
    din("W1T", (C1_IN, C1_OUT))
    din("W2T", (C1_OUT, C2_OUT))
    din("W3T", (C2_OUT, C3_OUT))
    din("Wg1T", (C3_OUT, G1_OUT), F16)
    din("Wg2T", (G1_OUT, G2_OUT), F16)
    din("W4Tp", (128, 8 * C4_OUT), F16)  # K-chunk j at cols [512j:512j+512]
    for nm, c in [("b1", 64), ("gm1", 64), ("bt1", 64), ("b2", 64), ("gm2", 64),
                  ("bt2", 64), ("b3", 64), ("gm3", 64), ("bt3", 64),
                  ("bg1", 128), ("gmg1", 128), ("btg1", 128)]:
        din(nm, (c, 1))
    # 1024-channel vectors as (128, 8): col j = channels [128j, 128j+128)
    for nm in ("bg2", "gmg2", "btg2"):
        din(nm, (128, 8))
    # 512-channel vectors as (128, 4)
    for nm in ("b4", "gm4", "bt4"):
        din(nm, (128, 4))

    out_t = nc.dram_tensor("out", [4, 128], F32, kind="ExternalOutput")

    with tile.TileContext(nc) as tc:
        with (
            tc.tile_pool(name="const", bufs=1) as constp,
            tc.tile_pool(name="persist", bufs=1) as pers,
            tc.tile_pool(name="dram", bufs=1, space="DRAM") as dram,
            tc.tile_pool(name="stats", bufs=2) as statp,
            tc.tile_pool(name="vec", bufs=4) as vecp,
        ):
            ident = constp.tile([128, 128], F32, tag="ident")
            make_identity(nc, ident[:])
            identh = constp.tile([128, 128], F16, tag="identh")
            make_identity(nc, identh[:])

            # ---- load params into SBUF ----
            def load(name, shape, dt=F32, pool=constp):
                t = pool.tile(list(shape), dt, tag=name)
                nc.sync.dma_start(t[:], inp[name][:])
                return t

            Lt1 = load("Lt1", (P1_ROWS, N), F16)
            Rt1 = load("Rt1", (P1_ROWS, N), F16)
            W1T = load("W1T", (C1_IN, C1_OUT))
            W2T = load("W2T", (C1_OUT, C2_OUT))
            W3T = load("W3T", (C2_OUT, C3_OUT))
            Wg1T = load("Wg1T", (C3_OUT, G1_OUT), F16)
            Wg2T = load("Wg2T", (G1_OUT, G2_OUT), F16)
            W4Tp = load("W4Tp", (128, 8 * C4_OUT), F16)
            pv = {nm: load(nm, (64, 1)) for nm in
                  ("b1", "gm1", "bt1", "b2", "gm2", "bt2", "b3", "gm3", "bt3")}
            pv.update({nm: load(nm, (128, 1)) for nm in ("bg1", "gmg1", "btg1")})
            pv.update({nm: load(nm, (128, 8)) for nm in ("bg2", "gmg2", "btg2")})
            pv.update({nm: load(nm, (128, 4)) for nm in ("b4", "gm4", "bt4")})

            ones128 = constp.tile([128, 1], F32, tag="ones128")
            nc.vector.memset(ones128[:], 1.0)
            ones_rowh = constp.tile([2, 128], F16, tag="ones_rowh")
            nc.vector.memset(ones_rowh[:], 1.0)
            negones = constp.tile([128, 1], F32, tag="negones")
            nc.vector.memset(negones[:], -1.0)
            onesh = constp.tile([128, 1], F16, tag="onesh")
            nc.vector.memset(onesh[:], 1.0)
            # one shared gpsimd register for every dma_gather's num_idxs
            # (to_reg allocates a fresh register per call otherwise)
            nidx_reg = nc.gpsimd.to_reg(N)

            # persistent activations
            hT = pers.tile([12, N], F32, tag="hT")
            h4 = pers.tile([64, N], F32, tag="h4")
            h5 = pers.tile([128, N], F16, tag="h5")
            mT = pers.tile([64, N], F16, tag="mT")
            m2T = pers.tile([128, N], F16, tag="m2T")

            g1t_dt = F32 if USE_DMA_GATHER else F16
            ftbl = dram.tile([N, 64], g1t_dt, tag="ftbl")
            ft2bl = dram.tile([N, 128], F16, tag="ft2bl")

            # ---------------- BN helper ----------------
            coll_seq = [0]

            def allreduce_stats(stats):
                """AllReduce a (128, 16) f32 stats tile across all cores."""
                i = coll_seq[0]
                coll_seq[0] += 1
                in_b = dram.tile([128, 16], F32, tag=f"arin{i}")
                out_b = dram.tile([128, 16], F32, tag=f"arout{i}")
                nc.sync.dma_start(in_b[:], stats[:])
                nc.gpsimd.collective_compute(
                    "AllReduce", OP.add,
                    replica_groups=[list(range(N_CORES))],
                    ins=[in_b.opt()], outs=[out_b.opt()],
                )
                back = statp.tile([128, 16], F32, tag="arback")
                nc.sync.dma_start(back[:], out_b[:])
                return back

            def bn_vectors(gst, col_s, col_q, gm_ap, bt_ap, c):
                """From summed stats -> (a, sh) APs of shape (c,1)."""
                v = vecp.tile([128, 8], F32, tag="bnv")
                s = gst[0:c, col_s : col_s + 1]
                q = gst[0:c, col_q : col_q + 1]
                nc.vector.tensor_scalar_mul(v[0:c, 0:1], s, INV_M)            # mu
                nc.vector.tensor_scalar_mul(v[0:c, 1:2], q, INV_M)            # E[y^2]
                nc.vector.tensor_mul(v[0:c, 2:3], v[0:c, 0:1], v[0:c, 0:1])   # mu^2
                nc.vector.tensor_sub(v[0:c, 1:2], v[0:c, 1:2], v[0:c, 2:3])   # var
                nc.vector.tensor_scalar_add(v[0:c, 1:2], v[0:c, 1:2], BN_EPS)
                nc.scalar.sqrt(v[0:c, 2:3], v[0:c, 1:2])                      # std
                nc.vector.reciprocal(v[0:c, 3:4], v[0:c, 2:3])                # 1/std
                nc.vector.tensor_mul(v[0:c, 4:5], gm_ap, v[0:c, 3:4])         # a
                nc.vector.tensor_mul(v[0:c, 5:6], v[0:c, 0:1], v[0:c, 4:5])   # mu*a
                nc.vector.tensor_sub(v[0:c, 6:7], bt_ap, v[0:c, 5:6])         # sh
                return v

            def wrap_idx(idx, smp, dramp, first):
                """(128,16) u32 neighbor idx -> (128,128) i16 wrapped layout
                for dma_gather (rows 0..15 hold idx of linear pos s*128+p at
                [p%16, 8s+p//16]); via a DRAM round-trip shuffle.

                Rows 16..127 are never read by the gather ucode but the sim
                bounds-checks them; zero them once per ring slot (the shuffle
                DMA fully rewrites rows 0..15 on reuse)."""
                idx16 = smp.tile([128, 16], I16, tag="idx16")
                nc.vector.tensor_copy(idx16[:], idx[:])
                scratch = dramp.tile([128, 16], I16, tag="iscr")
                nc.sync.dma_start(scratch[:], idx16[:])
                xw = smp.tile([128, 128], I16, tag="xw")
                if first:
                    nc.gpsimd.memset(xw[:], 0)
                dv = xw[0:16, :].rearrange("a (s b) -> a s b", b=8)
                srcv = scratch[:].rearrange("(b a) s -> a s b", b=8)
                nc.sync.dma_start(dv, srcv)
                return xw

            # ---------------- phase 1: knn on xyz -> covariance features ----
            # Gather-free: after top-16 VALUES (max8/mr8/max8), the selection
            # mask M[p,j] = (ut[p,j] >= v16[p]) is built in one DVE pass; the
            # neighbor sums [S(3), SS(9)] come from PE matmuls M^T-chunk @ P12
            # (hi/lo fp16 product table), so no indirect DMAs and no
            # FIND_INDEX8 passes at all. cov = SS - S S^T/16.
            P12sb = constp.tile([128, 16 * 24], F16, tag="P12sb")
            nc.sync.dma_start(
                P12sb[:].rearrange("p (c f) -> p c f", f=24),
                inp["P12"][:].rearrange("(c p) f -> p c f", p=128))
            with (
                tc.tile_pool(name="k1_psum", bufs=1, space="PSUM") as up,
                tc.tile_pool(name="k1_tp", bufs=2, space="PSUM") as tpp,
                tc.tile_pool(name="k1_cb", bufs=2, space="PSUM") as cbp,
                tc.tile_pool(name="k1_ut", bufs=2) as utp,
                tc.tile_pool(name="k1_scr", bufs=2) as scrp,
                tc.tile_pool(name="k1_small", bufs=3) as smp,
                tc.tile_pool(name="k1_mt", bufs=2) as mtp,
            ):
                def p1_topk(i):
                    """distmat + top-16 values + selection mask (DVE)."""
                    pu = up.tile([128, N], F32, tag="u")
                    for j in range(4):
                        nc.tensor.matmul(pu[:, ts(j, 512)], Lt1[:, ts(i, 128)],
                                         Rt1[:, ts(j, 512)])
                    ut = utp.tile([128, N], F32, tag="ut")
                    nc.scalar.activation(ut[:], pu[:], AF.Identity)
                    m8 = smp.tile([128, 16], F32, tag="m8")
                    scr = scrp.tile([128, N], F32, tag="scr")
                    nc.vector.max(m8[:, 0:8], ut[:])
                    nc.vector.match_replace(scr[:], m8[:, 0:8], ut[:], NEG)
                    nc.vector.max(m8[:, 8:16], scr[:])
                    # selection mask (0/1 fp16) via a second match_replace
                    # (first-occurrence replace == reference lowest-index
                    # tie-break): scr2 has all top-16 = NEG, so ut - scr2 is
                    # 0 for non-selected and ~1e30 for selected; min(.,1)
                    # gives exact 0/1.  (tensor_scalar is_ge miscompares on
                    # HW DVE despite simulating correctly.)
                    scr2 = scrp.tile([128, N], F32, tag="scr2")
                    nc.vector.match_replace(scr2[:], m8[:, 8:16], scr[:], NEG)
                    dif = scrp.tile([128, N], F32, tag="dif")
                    nc.vector.tensor_sub(dif[:], ut[:], scr2[:])
                    Mm = scrp.tile([128, N], F16, tag="Mm")
                    nc.vector.tensor_scalar_min(Mm[:], dif[:], 1.0)
                    xt = smp.tile([128, 4], F32, tag="xt")
                    nc.sync.dma_start(xt[:], inp["xpad"][ts(i, 128), 0:4])
                    return Mm, xt

                def p1_maskmm(i, Mm):
                    """[S,SS] += (M^T chunk) @ P12 on PE (lags topk by one
                    block so the PE queue never stalls the next distmat)."""
                    pcb = cbp.tile([128, 24], F32, tag="pcb")
                    MT = mtp.tile([128, N], F16, tag="MT")
                    for c in range(NB):
                        mtps = tpp.tile([128, 128], F16, tag="mtps")
                        nc.tensor.transpose(mtps[:], Mm[:, ts(c, 128)],
                                            identh[:])
                        nc.scalar.activation(MT[:, ts(c, 128)], mtps[:],
                                             AF.Identity)
                        nc.tensor.matmul(pcb[:], MT[:, ts(c, 128)],
                                         P12sb[:, 24 * c : 24 * c + 24],
                                         start=(c == 0), stop=(c == NB - 1))
                    return pcb

                def p1_cov(i, pcb, xt):
                    """covariance features for block i from [S,SS] sums."""
                    s24 = smp.tile([128, 24], F32, tag="s24")
                    nc.scalar.activation(s24[:], pcb[:], AF.Identity)
                    cb = smp.tile([128, 12], F32, tag="cb")
                    nc.vector.tensor_copy(cb[:, 0:3], xt[:, 0:3])
                    # hi+lo halves -> S (cols 0:3), SS (cols 3:12)
                    nc.vector.tensor_add(s24[:, 0:12], s24[:, 0:12],
                                         s24[:, 12:24])
                    nc.vector.tensor_copy(cb[:, 3:12], s24[:, 3:12])
                    # subtract S_c*S_d/16
                    m3 = smp.tile([128, 3], F32, tag="m3")
                    nc.vector.tensor_scalar_mul(m3[:], s24[:, 0:3], 0.25)
                    p16 = smp.tile([128, 9], F32, tag="p16")
                    for c in range(3):
                        nc.vector.tensor_scalar_mul(
                            p16[:, 3 * c : 3 * c + 3], m3[:], m3[:, c : c + 1])
                    nc.vector.tensor_sub(cb[:, 3:12], cb[:, 3:12], p16[:])
                    # transpose (128, 12) -> (12, 128) into hT, via fp16 so
                    # the transpose shares the mtps PSUM slots (bank budget)
                    cb16 = smp.tile([128, 12], F16, tag="cb16")
                    nc.scalar.activation(cb16[:], cb[:], AF.Identity)
                    ptp = tpp.tile([128, 128], F16, tag="mtps")
                    nc.tensor.transpose(ptp[0:12, :], cb16[:], identh[:])
                    nc.scalar.activation(hT[0:12, ts(i, 128)], ptp[0:12, :],
                                         AF.Identity)

                # 2-deep software pipeline: topk(i) | maskmm(i-1) | cov(i-2)
                mask_pend = None
                cov_pend = None
                for i in range(NB):
                    mk = p1_topk(i)
                    if cov_pend is not None:
                        p1_cov(*cov_pend)
                        cov_pend = None
                    if mask_pend is not None:
                        im, Mm_, xt_ = mask_pend
                        cov_pend = (im, p1_maskmm(im, Mm_), xt_)
                    mask_pend = (i, *mk)
                if cov_pend is not None:
                    p1_cov(*cov_pend)
                im, Mm_, xt_ = mask_pend
                p1_cov(im, p1_maskmm(im, Mm_), xt_)
            # ---------------- phase 2: conv1..conv3 ----------------
            scr2 = pers.tile([128, N], F32, tag="sqscr")

            def conv_bn_small(rhs_ap, WT, cin, cout, b, gm, bt, h_out,
                              gst_pre=None):
                with tc.tile_pool(name="conv_psum", bufs=1, space="PSUM") as cp:
                    py = cp.tile([cout, N], F32, tag="y")
                    for j in range(4):
                        nc.tensor.matmul(py[:, ts(j, 512)], WT[:],
                                         rhs_ap[:, ts(j, 512)])
                    hpre = h_out[0:cout, :]
                    if gst_pre is None:
                        stats = statp.tile([128, 16], F32, tag="st")
                        nc.vector.memset(stats[:], 0.0)
                        nc.scalar.activation(hpre, py[:], AF.Identity,
                                             bias=b[0:cout, 0:1],
                                             accum_out=stats[0:cout, 0:1])
                        nc.scalar.activation(scr2[0:cout, :], hpre, AF.Square,
                                             accum_out=stats[0:cout, 1:2])
                        gst = allreduce_stats(stats)
                    else:
                        # stats precomputed analytically; AR already in
                        # flight and overlaps these matmuls/copies
                        nc.scalar.activation(hpre, py[:], AF.Identity,
                                             bias=b[0:cout, 0:1])
                        gst = gst_pre
                    v = bn_vectors(gst, 0, 1, gm[0:cout, 0:1], bt[0:cout, 0:1], cout)
                    nc.scalar.activation(hpre, hpre, AF.Relu,
                                         scale=v[0:cout, 4:5], bias=v[0:cout, 6:7])

            def analytic_stats(momt, W32, b, cin, cout, tag):
                """BN stats from moment sums. momt: PSUM AP [cin, cin+1] =
                [M | s] with M = sum_n f f^T, s = sum_n f (over this core's
                n=N points). stats col0 = W s + N b, col1 = diag(W M W^T) +
                2 b (W s) + N b^2.  Returns the in-flight AR'd stats tile."""
                stats = statp.tile([128, 16], F32, tag=f"ast{tag}")
                nc.vector.memset(stats[:], 0.0)
                with (
                    tc.tile_pool(name=f"as_ps{tag}", bufs=1, space="PSUM") as aps,
                    tc.tile_pool(name=f"as_sb{tag}", bufs=1) as asb,
                ):
                    Ms = asb.tile([cin, cin + 1], F32, tag="Ms")
                    nc.scalar.activation(Ms[:], momt, AF.Identity)
                    pP = aps.tile([cin, cout], F32, tag="pP")
                    nc.tensor.matmul(pP[:], Ms[:, 0:cin], W32[:])
                    Q32 = asb.tile([cin, cout], F32, tag="Q32")
                    nc.vector.tensor_mul(Q32[:], pP[:], W32[:])
                    pq = aps.tile([cout, 2], F32, tag="pq")
                    nc.tensor.matmul(pq[:, 0:1], Q32[:], ones128[0:cin, :])
                    nc.tensor.matmul(pq[:, 1:2], W32[:], Ms[:, cin : cin + 1])
                    qs = asb.tile([cout, 2], F32, tag="qs")
                    nc.scalar.activation(qs[:], pq[:], AF.Identity)
                    t4 = vecp.tile([128, 1], F32, tag=f"ast4{tag}")
                    bc = b[0:cout, 0:1]
                    nc.vector.tensor_scalar_mul(t4[0:cout, :], bc, float(N))
                    nc.vector.tensor_add(stats[0:cout, 0:1], qs[:, 1:2],
                                         t4[0:cout, :])
                    nc.vector.tensor_mul(t4[0:cout, :], bc, qs[:, 1:2])
                    nc.vector.tensor_scalar_mul(t4[0:cout, :], t4[0:cout, :], 2.0)
                    nc.vector.tensor_add(stats[0:cout, 1:2], qs[:, 0:1],
                                         t4[0:cout, :])
                    nc.vector.tensor_mul(t4[0:cout, :], bc, bc)
                    nc.vector.tensor_scalar_mul(t4[0:cout, :], t4[0:cout, :],
                                                float(N))
                    nc.vector.tensor_add(stats[0:cout, 1:2],
                                         stats[0:cout, 1:2], t4[0:cout, :])
                return allreduce_stats(stats)

            with tc.tile_pool(name="hpre_pool", bufs=2) as scrp2:
                conv_bn_small(hT[:], W1T, C1_IN, 64, pv["b1"], pv["gm1"],
                              pv["bt1"], h4)
                conv_bn_small(h4[:], W2T, 64, 64, pv["b2"], pv["gm2"], pv["bt2"], h4)
                conv_bn_small(h4[:], W3T, 64, 64, pv["b3"], pv["gm3"], pv["bt3"], h4)

                # feature table (N, 64) f32 for g1 dma_gather (256B rows)
                with tc.tile_pool(name="ft_psum", bufs=2, space="PSUM") as ftp:
                    for i in range(NB):
                        ptp = ftp.tile([128, 64], F32, tag="ftT")
                        nc.tensor.transpose(ptp[:], h4[:, ts(i, 128)],
                                            ident[0:64, 0:64])
                        ft = scrp2.tile([128, 64], g1t_dt, tag="fts")
                        nc.scalar.activation(ft[:], ptp[:], AF.Identity)
                        nc.sync.dma_start(ftbl[ts(i, 128), :], ft[:])

            # ---------------- graph layer helper ----------------
            def graph_knn(feat, cdim, ftable, pooled_T, g_dt, mom=None):
                """kNN in feature space + gather + max-pool; writes pooled^T
                (cdim, N) fp16 into pooled_T. feat: (cdim, N) f32 or fp16.

                Distance matmuls run on fp16 inputs with f32 PSUM accum; the
                -|f|^2 column term rides as fp16 hi/lo contraction rows (fold
                into one matmul when cdim+2 <= 128, else a separate 2-row
                matmul); the -|f|^2 row term is a f32 per-partition bias at the
                PSUM->SBUF copy, recentering ut to -d. Top-k compare is f32."""
                fold = cdim + 2 <= 128
                with tc.tile_pool(name="gk_sb", bufs=1) as sb:
                    with tc.tile_pool(name="gk_prep", bufs=1, space="PSUM") as pp:
                        # aa[n] = sum_c feat[c,n]^2  (via ones-vector matmul)
                        nc.scalar.activation(scr2[0:cdim, :], feat[:], AF.Square)
                        pa = pp.tile([1, N], F32, tag="aa")
                        for j in range(4):
                            nc.tensor.matmul(pa[:, ts(j, 512)], ones128[0:cdim, :],
                                             scr2[0:cdim, ts(j, 512)])
                        # (no per-partition -|f_p|^2 recentering: a
                        # per-partition constant cannot change that row's
                        # top-k selection, and compare values are f32)
                        # hi/lo fp16 split of -aa for the matmul free-axis term
                        hi_h = sb.tile([1, N], F16, tag="hi_h")
                        nc.scalar.activation(hi_h[:], pa[:], AF.Identity,
                                             scale=-1.0)
                        hi_f = sb.tile([1, N], F32, tag="hi_f")
                        nc.scalar.activation(hi_f[:], hi_h[:], AF.Identity)
                        naa_f = sb.tile([1, N], F32, tag="naa_f")
                        nc.scalar.activation(naa_f[:], pa[:], AF.Identity,
                                             scale=-1.0)
                        lo_h = sb.tile([1, N], F16, tag="lo_h")
                        nc.vector.tensor_sub(lo_h[:], naa_f[:], hi_f[:])
                        if fold:
                            Lt = sb.tile([cdim + 2, N], F16, tag="lt")
                            Rt = sb.tile([cdim + 2, N], F16, tag="rt")
                            nc.scalar.activation(Lt[0:cdim, :], feat[:],
                                                 AF.Identity, scale=2.0)
                            nc.vector.memset(Lt[cdim : cdim + 2, :], 1.0)
                            nc.scalar.activation(Rt[0:cdim, :], feat[:], AF.Identity)
                            nc.sync.dma_start(Rt[cdim : cdim + 1, :], hi_h[:])
                            nc.sync.dma_start(Rt[cdim + 1 : cdim + 2, :], lo_h[:])
                        else:
                            Lt = sb.tile([cdim, N], F16, tag="lt")
                            nc.scalar.activation(Lt[:], feat[:], AF.Identity,
                                                 scale=2.0)
                            aarows = sb.tile([2, N], F16, tag="aarows")
                            nc.sync.dma_start(aarows[0:1, :], hi_h[:])
                            nc.sync.dma_start(aarows[1:2, :], lo_h[:])
                            feat_h = feat

                    with (
                        tc.tile_pool(name="gk_psum", bufs=1, space="PSUM") as up,
                        tc.tile_pool(name="gk_tp", bufs=2, space="PSUM") as tpp,
                        tc.tile_pool(name="gk_ut", bufs=2) as utp,
                        tc.tile_pool(name="gk_scr", bufs=2) as scrp,
                        tc.tile_pool(name="gk_small", bufs=3) as smp,
                        tc.tile_pool(name="gk_g", bufs=4) as gp,
                        tc.tile_pool(name="gk_dram", bufs=3, space="DRAM") as dramp,
                    ):
                        def gk_stage_a(i):
                            """top-k + gather launch for block i."""
                            pu = up.tile([128, N], F32, tag="u")
                            for j in range(4):
                                if fold:
                                    nc.tensor.matmul(pu[:, ts(j, 512)],
                                                     Lt[:, ts(i, 128)],
                                                     Rt[:, ts(j, 512)])
                                else:
                                    nc.tensor.matmul(pu[:, ts(j, 512)],
                                                     Lt[:, ts(i, 128)],
                                                     feat_h[:, ts(j, 512)],
                                                     start=True, stop=False)
                                    nc.tensor.matmul(pu[:, ts(j, 512)],
                                                     ones_rowh[:, 0:128],
                                                     aarows[:, ts(j, 512)],
                                                     start=False, stop=True)
                            ut = utp.tile([128, N], F32, tag="ut")
                            nc.scalar.activation(ut[:], pu[:], AF.Identity)
                            m8 = smp.tile([128, 16], F32, tag="m8")
                            scr = scrp.tile([128, N], F32, tag="scr")
                            nc.vector.max(m8[:, 0:8], ut[:])
                            nc.vector.match_replace(scr[:], m8[:, 0:8], ut[:], NEG)
                            nc.vector.max(m8[:, 8:16], scr[:])
                            idx = smp.tile([128, 16], U32, tag="idx")
                            nc.vector.max_index(idx[:, 0:8], m8[:, 0:8], ut[:])
                            nc.vector.max_index(idx[:, 8:16], m8[:, 8:16], ut[:])

                            g = gp.tile([128, 16 * cdim], g_dt, tag="g")
                            if USE_DMA_GATHER:
                                xw = wrap_idx(idx, smp, dramp, i < 3)
                                nc.gpsimd.dma_gather(
                                    g[:].rearrange("p (s c) -> p s c", c=cdim),
                                    ftable[:], xw[:], num_idxs=N,
                                    num_idxs_reg=nidx_reg, elem_size=cdim)
                            else:
                                for sx in range(16):
                                    nc.gpsimd.indirect_dma_start(
                                        g[:, cdim * sx : cdim * (sx + 1)], None,
                                        ftable[:],
                                        IndirectOffsetOnAxis(
                                            ap=idx[:, sx : sx + 1], axis=0))
                            return g

                        def gk_stage_b(i, g):
                            """max-pool + transpose for block i."""
                            w = 8 * cdim
                            nc.vector.tensor_tensor(g[:, 0:w], g[:, 0:w],
                                                    g[:, w : 2 * w], op=OP.max)
                            w //= 2
                            nc.vector.tensor_tensor(g[:, 0:w], g[:, 0:w],
                                                    g[:, w : 2 * w], op=OP.max)
                            w //= 2
                            nc.vector.tensor_tensor(g[:, 0:w], g[:, 0:w],
                                                    g[:, w : 2 * w], op=OP.max)
                            w //= 2
                            nc.vector.tensor_tensor(g[:, 0:w], g[:, 0:w],
                                                    g[:, w : 2 * w], op=OP.max)
                            if mom is not None:
                                # fused moment sums [M | s] += g^T [g | 1] on
                                # PE -- ONE accumulation chain (two
                                # interleaved open chains in one PSUM bank
                                # corrupt each other on HW)
                                nc.vector.memset(g[:, cdim : cdim + 1], 1.0)
                                nc.tensor.matmul(mom, g[:, 0:cdim],
                                                 g[:, 0 : cdim + 1],
                                                 start=(i == 0),
                                                 stop=(i == NB - 1))
                            if g_dt == F16:
                                ptp = tpp.tile([cdim, 128], F16, tag="plT")
                                nc.tensor.transpose(ptp[:], g[:, 0:cdim],
                                                    identh[:])
                            else:
                                ptp = tpp.tile([cdim, 128], F32, tag="plTf")
                                nc.tensor.transpose(ptp[:], g[:, 0:cdim],
                                                    ident[:])
                            nc.scalar.activation(pooled_T[:, ts(i, 128)], ptp[:],
                                                 AF.Identity)

                        # software-pipelined: B lags A by two iterations so
                        # the DVE stream never waits on a gather's completion
                        pend = []
                        for i in range(NB):
                            pend.append((i, gk_stage_a(i)))
                            if len(pend) > 2:
                                ib, g_ = pend.pop(0)
                                gk_stage_b(ib, g_)
                        for ib, g_ in pend:
                            gk_stage_b(ib, g_)

            # ---------------- phase 3: graph layer 1 ----------------
            Wg1T32 = constp.tile([64, 128], F32, tag="Wg1T32")
            nc.scalar.activation(Wg1T32[:], Wg1T[:], AF.Identity)
            with tc.tile_pool(name="g1mom", bufs=1, space="PSUM") as momg1:
                momtg1 = momg1.tile([64, 65], F32, tag="momtg1")
                graph_knn(h4, 64, ftbl, mT, g1t_dt,
                          mom=momtg1[0:64, 0:65])
                gstg1 = analytic_stats(momtg1[0:64, :], Wg1T32, pv["bg1"],
                                       64, 128, "g1")
            with tc.tile_pool(name="hpre_pool2", bufs=2) as scrp2b:
                conv_bn_small(mT[:], Wg1T, 64, 128, pv["bg1"], pv["gmg1"],
                              pv["btg1"], h5, gst_pre=gstg1)
                with tc.tile_pool(name="ft2_psum", bufs=2, space="PSUM") as ftp:
                    for i in range(NB):
                        ptp = ftp.tile([128, 128], F16, tag="ft2T")
                        nc.tensor.transpose(ptp[:], h5[:, ts(i, 128)], identh[:])
                        ft = scrp2b.tile([128, 128], F16, tag="ft2s")
                        nc.scalar.activation(ft[:], ptp[:], AF.Identity)
                        nc.sync.dma_start(ft2bl[ts(i, 128), :], ft[:])

            # ---------------- phase 4: graph layer 2 + convg2 ----------------
            # BN stats for convg2 come analytically from the pooled features'
            # moment sums, accumulated on the (idle) PE during the knn phase:
            #   sum_n y      = W s + n b
            #   sum_n y^2    = diag(W M W^T) + 2 b (W s) + n b^2
            # so the stats AllReduce fires right at phase end and overlaps the
            # convg2 matmuls; the 8 per-mblk SQUARE passes disappear.
            stats = statp.tile([128, 16], F32, tag="stg2")
            with tc.tile_pool(name="g2mom", bufs=1, space="PSUM") as momp:
                mom_Ms = momp.tile([128, 129], F32, tag="Ms2")
                graph_knn(h5, 128, ft2bl, m2T, F16,
                          mom=mom_Ms[0:128, 0:129])

                with (
                    tc.tile_pool(name="g2st_psum", bufs=1, space="PSUM") as sp2,
                    tc.tile_pool(name="g2st_sb", bufs=1) as sb2,
                ):
                    # hi/lo fp16 split of M (values up to ~n, fp16 alone
                    # would quantize at ~5e-4; hi/lo keeps ~f32)
                    M32 = sb2.tile([128, 128], F32, tag="M32")
                    nc.scalar.activation(M32[:], mom_Ms[:, 0:128], AF.Identity)
                    Mh = sb2.tile([128, 128], F16, tag="Mh")
                    nc.scalar.activation(Mh[:], M32[:], AF.Identity)
                    Mhf = sb2.tile([128, 128], F32, tag="Mhf")
                    nc.scalar.activation(Mhf[:], Mh[:], AF.Identity)
                    Ml = sb2.tile([128, 128], F16, tag="Ml")
                    nc.vector.tensor_sub(Ml[:], M32[:], Mhf[:])
                    sh = sb2.tile([128, 1], F16, tag="sh")
                    nc.scalar.activation(sh[:], mom_Ms[:, 128:129], AF.Identity)
                    # f32 copy of Wg2T for the Hadamard step
                    W32 = sb2.tile([128, 8 * 128], F32, tag="W32")
                    nc.scalar.activation(W32[:], Wg2T[:], AF.Identity)

                    psy = sp2.tile([128, 8], F32, tag="psy")
                    pq = sp2.tile([128, 8], F32, tag="pq")
                    for mb in range(8):
                        Wmb = Wg2T[:, ts(mb, 128)]
                        pP = sp2.tile([128, 128], F32, tag="pP")
                        nc.tensor.matmul(pP[:], Mh[:], Wmb,
                                         start=True, stop=False)
                        nc.tensor.matmul(pP[:], Ml[:], Wmb,
                                         start=False, stop=True)
                        Q32 = sb2.tile([128, 128], F32, tag="Q32")
                        nc.vector.tensor_mul(Q32[:], pP[:],
                                             W32[:, ts(mb, 128)])
                        nc.tensor.matmul(pq[:, mb : mb + 1], Q32[:],
                                         ones128[:])
                        nc.tensor.matmul(psy[:, mb : mb + 1], Wmb, sh[:])
                    syv = sb2.tile([128, 8], F32, tag="syv")
                    nc.scalar.activation(syv[:], psy[:], AF.Identity)
                    qv = sb2.tile([128, 8], F32, tag="qv")
                    nc.scalar.activation(qv[:], pq[:], AF.Identity)
                    # stats[:, 0:8] = syv + n*b ; stats[:, 8:16] =
                    #   qv + 2 b syv + n b^2   (n = N points per core)
                    bg2 = pv["bg2"]
                    t8 = sb2.tile([128, 8], F32, tag="t8")
                    nc.vector.tensor_scalar_mul(t8[:], bg2[:], float(N))
                    nc.vector.tensor_add(stats[:, 0:8], syv[:], t8[:])
                    nc.vector.tensor_mul(t8[:], bg2[:], syv[:])
                    nc.vector.tensor_scalar_mul(t8[:], t8[:], 2.0)
                    nc.vector.tensor_add(stats[:, 8:16], qv[:], t8[:])
                    nc.vector.tensor_mul(t8[:], bg2[:], bg2[:])
                    nc.vector.tensor_scalar_mul(t8[:], t8[:], float(N))
                    nc.vector.tensor_add(stats[:, 8:16], stats[:, 8:16],
                                         t8[:])

            gst = allreduce_stats(stats)

            # convg2: (1024, 128) @ (128, N) -- matmuls overlap the AllReduce
            latep_cm = tc.tile_pool(name="late", bufs=1)
            latep = latep_cm.__enter__()
            h6 = [latep.tile([128, N], F16, tag=f"h6_{j}", name=f"h6_{j}")
                  for j in range(8)]
            with tc.tile_pool(name="g2conv_psum", bufs=2, space="PSUM") as cp:
                for mblk in range(8):
                    py = cp.tile([128, N], F32, tag="y")
                    for j in range(4):
                        nc.tensor.matmul(py[:, ts(j, 512)],
                                         Wg2T[:, ts(mblk, 128)],
                                         m2T[:, ts(j, 512)])
                    nc.scalar.activation(h6[mblk][:], py[:], AF.Identity,
                                         bias=pv["bg2"][:, mblk : mblk + 1])
                for mblk in range(8):
                    v = bn_vectors(gst, mblk, 8 + mblk,
                                   pv["gmg2"][:, mblk : mblk + 1],
                                   pv["btg2"][:, mblk : mblk + 1], 128)
                    nc.scalar.activation(h6[mblk][:], h6[mblk][:], AF.Relu,
                                         scale=v[:, 4:5], bias=v[:, 6:7])

            # ---------------- phase 5: conv4 + BN4 + global max ----------------
            with (
                tc.tile_pool(name="c4_psum", bufs=2, space="PSUM") as cp,
                tc.tile_pool(name="c4_sb", bufs=2) as hp,
            ):
                stats = statp.tile([128, 16], F32, tag="st4")
                nc.vector.memset(stats[:], 0.0)
                maxc = pers.tile([128, 4], F32, tag="maxc")
                for mblk in range(4):
                    py = cp.tile([128, N], F32, tag="y")
                    for j in range(4):
                        for k in range(8):
                            nc.tensor.matmul(
                                py[:, ts(j, 512)],
                                W4Tp[:, 512 * k + 128 * mblk : 512 * k + 128 * mblk + 128],
                                h6[k][:, ts(j, 512)],
                                start=(k == 0), stop=(k == 7))
                    y4 = hp.tile([128, N], F32, tag="y4")
                    nc.scalar.activation(y4[:], py[:], AF.Identity,
                                         bias=pv["b4"][:, mblk : mblk + 1],
                                         accum_out=stats[:, mblk : mblk + 1])
                    nc.scalar.activation(scr2[:], y4[:], AF.Square,
                                         accum_out=stats[:, 8 + mblk : 9 + mblk])
                    nc.vector.reduce_max(maxc[:, mblk : mblk + 1], y4[:], axis=AX.X)
                gst = allreduce_stats(stats)
                out4 = pers.tile([128, 4], F32, tag="out4")
                for mblk in range(4):
                    v = bn_vectors(gst, mblk, 8 + mblk,
                                   pv["gm4"][:, mblk : mblk + 1],
                                   pv["bt4"][:, mblk : mblk + 1], 128)
                    # out = (max - mu) * a + bt  (valid since gm>0)
                    nc.vector.tensor_sub(out4[:, mblk : mblk + 1],
                                         maxc[:, mblk : mblk + 1], v[:, 0:1])
                    nc.vector.tensor_mul(out4[:, mblk : mblk + 1],
                                         out4[:, mblk : mblk + 1], v[:, 4:5])
                    nc.vector.tensor_add(out4[:, mblk : mblk + 1],
                                         out4[:, mblk : mblk + 1],
                                         pv["bt4"][:, mblk : mblk + 1])
            with (
                tc.tile_pool(name="fin_psum", bufs=1, space="PSUM") as fp,
                tc.tile_pool(name="fin_sb", bufs=1) as fsb,
            ):
                ptp = fp.tile([4, 128], F32, tag="outT")
                nc.tensor.transpose(ptp[:], out4[:], ident[:])
                outs = fsb.tile([4, 128], F32, tag="outs")
                nc.scalar.activation(outs[:], ptp[:], AF.Identity)
                nc.sync.dma_start(out_t[:], outs[:])
            latep_cm.__exit__(None, None, None)

    # auto-insert gpsimd library reloads (dma_gather lives in the mlp
    # library) and generate ISA bytes for the inserted MODIFY_POOL_CONFIG
    # instructions -- walrus rejects empty .instr with "ISA wrong length"
    inst_type_to_lib_mask = {}
    for lib in all_libraries:
        for it in lib.instructions:
            inst_type_to_lib_mask[it] = (
                inst_type_to_lib_mask.get(it, 0) | (1 << lib.index))
    bass_rust.insert_library_loads(
        nc, inst_type_to_lib_mask, len(all_libraries), standard.index)
    mybir.codegen_inst_isa_subclasses(nc)
    split_drain_waits(nc)
    return nc


_PROGRAM = None


def _get_program():
    global _PROGRAM
    if _PROGRAM is None:
        _PROGRAM = build_program()
    return _PROGRAM


def make_in_maps(x, weights):
    """x: (B, N, 3); weights: dict of the reference param arrays."""
    f16 = np.float16
    shared = {}
    shared["W1T"] = np.ascontiguousarray(weights["W1"].T)
    shared["W2T"] = np.ascontiguousarray(weights["W2"].T)
    shared["W3T"] = np.ascontiguousarray(weights["W3"].T)
    shared["Wg1T"] = np.ascontiguousarray(weights["Wg1"].T).astype(f16)
    shared["Wg2T"] = np.ascontiguousarray(weights["Wg2"].T).astype(f16)
    W4 = weights["W4"]
    chunks = [np.ascontiguousarray(W4[:, 128 * j : 128 * (j + 1)].T) for j in range(8)]
    shared["W4Tp"] = np.ascontiguousarray(np.concatenate(chunks, axis=1)).astype(f16)
    for nm in ("b1", "gm1", "bt1", "b2", "gm2", "bt2", "b3", "gm3", "bt3"):
        shared[nm] = np.ascontiguousarray(weights[nm].reshape(-1, 1))
    for nm in ("bg1", "gmg1", "btg1"):
        shared[nm] = np.ascontiguousarray(weights[nm].reshape(-1, 1))
    for nm in ("bg2", "gmg2", "btg2"):
        shared[nm] = np.ascontiguousarray(weights[nm].reshape(8, 128).T)
    for nm in ("b4", "gm4", "bt4"):
        shared[nm] = np.ascontiguousarray(weights[nm].reshape(4, 128).T)

    in_maps = []
    for c in range(B):
        xc = np.asarray(x[c], dtype=np.float32)       # (N, 3)
        xT = np.ascontiguousarray(xc.T)               # (3, N)
        aa = (xc * xc).sum(axis=1).astype(np.float32)  # (N,)
        m = dict(shared)
        # phase-1 distmat via exact fp16 hi/lo split:
        # 2x.x' = 2xh.x'h + 2xh.x'l + 2xl.x'h (+O(2^-22));  aa = aah + aal
        xh = xT.astype(f16)
        xl = (xT - xh.astype(np.float32)).astype(f16)
        aah = aa.astype(f16)
        aal = (aa - aah.astype(np.float32)).astype(f16)
        m["Lt1"] = np.ascontiguousarray(np.concatenate(
            [2.0 * xh, 2.0 * xh, 2.0 * xl,
             np.ones((2, N), f16)], axis=0).astype(f16))
        m["Rt1"] = np.ascontiguousarray(np.concatenate(
            [xh, xl, xh, -aah[None, :], -aal[None, :]], axis=0).astype(f16))
        m["xpad"] = np.ascontiguousarray(
            np.concatenate([xc, np.zeros((N, 1), np.float32)], axis=1))
        # product table for the mask-matmul covariance path: per point
        # [x(3), x_c*x_d(9)] split hi/lo fp16 -> (N, 24)
        prods = (xc[:, :, None] * xc[:, None, :]).reshape(N, 9)
        P = np.concatenate([xc, prods], axis=1).astype(np.float32)  # (N, 12)
        Ph = P.astype(f16)
        Pl = (P - Ph.astype(np.float32)).astype(f16)
        m["P12"] = np.ascontiguousarray(np.concatenate([Ph, Pl], axis=1))
        in_maps.append(m)
    return in_maps


def kernel(**inputs):
    x = np.asarray(inputs["x"], dtype=np.float32)
    weights = {k: np.asarray(v, dtype=np.float32)
               for k, v in inputs.items() if k != "x"}
    nc = _get_program()
    in_maps = make_in_maps(x, weights)
    res = run_bass_kernel_spmd(nc, in_maps, core_ids=list(range(N_CORES)),
                               trace=False)
    out = np.stack([res.results[c]["out"].reshape(512) for c in range(B)])
    return out.astype(np.float32)


if __name__ == "__main__":
    nc = build_program()
    print("program built ok")



# revision 23
# speedup vs baseline: 1.0051x; 1.0051x over previous
"""Trainium2 Bass kernel for nn_Encoder_60318520705555 (DGCNN-style encoder).

Sharding: data-parallel over batch B=8 across 8 NeuronCores (1 batch element
per core); BN batch statistics are all-reduced across cores (6 tiny
AllReduces). Everything else is core-local.

Self-contained: hardcodes shapes (B=8, N=2048, K=16, channel sizes).

Perf/accuracy design:
  - phase 1 (xyz knn -> covariance features) is GATHER-FREE: after the top-16
    VALUES (max8 / match_replace8 / max8 on DVE), a 0/1 fp16 selection mask
    M[p,j] = (ut[p,j] >= 16th value) is built in one DVE pass; neighbor sums
    [S(3), SS(9)] come from PE matmuls (M^T chunk) @ P12 against a hi/lo fp16
    product table, and cov = SS - S S^T/16.  This removes all 256 phase-1
    indirect DMAs (Pool) and both FIND_INDEX8 passes per block (DVE).
    (InstDMAGatherAnt and multi-index indirect DMAs both crash this runtime's
    Q7/SWDGE -- HW-tested -- so graph-layer gathers stay 16x single-index
    SWDGE indirect DMAs per block, the graph-phase cadence limit.)
  - all distance matmuls take fp16 inputs with f32 PSUM accumulation; phase-1
    uses an exact hi/lo split (11 contraction rows, error ~2^-22); the graph
    layers ride the "-|f|^2" free-axis term as fp16 hi/lo rows.  Top-k
    compare runs on f32 values (16-bit compare flips near-tied selections:
    measured 7e-2 bf16 / 9e-3 fp16 / 6e-4 f32).  DVE max8/match/find run at
    the same speed for f32 and fp16 (no 2x uop variants), so f32 is free.
  - activations, feature tables, gathers and the g1/g2/c4 conv weights are
    fp16 (value error only, ~1e-4..1e-3). Phase-1 geometry (covariances
    suffer catastrophic cancellation) and all BN statistics stay fp32.
"""

import sys

sys.path.insert(0, "/opt/trn_rl_repo")

import numpy as np

import bass_rust
import concourse.bass as bass
import concourse.mybir as mybir
import concourse.tile as tile
from concourse.bass import IndirectOffsetOnAxis
from concourse.bass_utils import run_bass_kernel_spmd
from concourse.masks import make_identity
from concourse.library_config import all_libraries, standard

F32 = mybir.dt.float32
F16 = mybir.dt.float16
U32 = mybir.dt.uint32
I16 = mybir.dt.int16
AF = mybir.ActivationFunctionType
AX = mybir.AxisListType
OP = mybir.AluOpType

# dma_gather (one Pool instruction per block) vs 16x indirect SWDGE DMAs
# (994ns fixed Pool cost each, but HW-validated)
USE_DMA_GATHER = False
P1_ROWS = 11  # phase-1 hi/lo fp16 distance matmul contraction rows

N_CORES = 8
B = 8
N = 2048
KNN = 16
NB = N // 128  # row blocks
BN_EPS = 1e-5
NEG = -1.0e30
INV_M = 1.0 / (B * N)  # BN mean divisor (global batch)

# conv layer channel sizes
C1_IN, C1_OUT = 12, 64
C2_OUT, C3_OUT = 64, 64
G1_OUT, G2_OUT = 128, 1024
C4_OUT = 512


def ts(i, s):
    return slice(i * s, (i + 1) * s)


def split_drain_waits(nc, limit=1):
    """walrus core_v3 codegen rejects instructions carrying more than one
    sync wait; hoist excess waits onto single-wait NoOp carriers just
    before the instruction (engine streams are in-order, so this is
    semantically equivalent)."""
    for f in nc.m.functions:
        for bb in f.blocks:
            out = []
            changed = False
            for inst in bb.instructions:
                si = inst.sync_info
                if si is not None and len(si.on_wait) > limit:
                    waits = list(si.on_wait)
                    chunks = [waits[i : i + limit] for i in range(0, len(waits), limit)]
                    for j, ch in enumerate(chunks[:-1]):
                        d = mybir.InstNoOp(name=f"{inst.name}-sw{j}", engine=inst.engine)
                        d.sync_info = bass_rust.SyncInfo(on_wait=ch, on_update=[])
                        nc.register_instruction(d, overwrite=True)
                        out.append(d)
                    si.on_wait = chunks[-1]
                    inst.sync_info = si
                    changed = True
                out.append(inst)
            if changed:
                bb.instructions = out


def build_program():
    nc = bass.Bass()

    # ---- I/O declarations (per-core shapes; host prepares the layouts) ----
    inp = {}

    def din(name, shape, dt=F32):
        inp[name] = nc.dram_tensor(name, list(shape), dt, kind="ExternalInput")
        return inp[name]

    # phase-1 distance matmul, exact via fp16 hi/lo split (error ~2^-22):
    # Lt1 = [2x_hi(3); 2x_hi(3); 2x_lo(3); 1; 1]
    # Rt1 = [x_hi(3);  x_lo(3);  x_hi(3); -aa_hi; -aa_lo]
    din("Lt1", (P1_ROWS, N), F16)
    din("Rt1", (P1_ROWS, N), F16)
    din("xpad", (N, 4))     # x padded to 4 cols (16B rows, self-x loads)
    din("P12", (N, 24), F16)  # [x(3), x_c*x_d(9)] hi/lo product table
    din("W1T", (C1_IN, C1_OUT))
    din("W2T", (C1_OUT, C2_OUT))
    din("W3T", (C2_OUT, C3_OUT))
    din("Wg1T", (C3_OUT, G1_OUT), F16)
    din("Wg2T", (G1_OUT, G2_OUT), F16)
    din("W4Tp", (128, 8 * C4_OUT), F16)  # K-chunk j at cols [512j:512j+512]
    for nm, c in [("b1", 64), ("gm1", 64), ("bt1", 64), ("b2", 64), ("gm2", 64),
                  ("bt2", 64), ("b3", 64), ("gm3", 64), ("bt3", 64),
                  ("bg1", 128), ("gmg1", 128), ("btg1", 128)]:
        din(nm, (c, 1))
    # 1024-channel vectors as (128, 8): col j = channels [128j, 128j+128)
    for nm in ("bg2", "gmg2", "btg2"):
        din(nm, (128, 8))
    # 512-channel vectors as (128, 4)
    for nm in ("b4", "gm4", "bt4"):
        din(nm, (128, 4))

    out_t = nc.dram_tensor("out", [4, 128], F32, kind="ExternalOutput")

    with tile.TileContext(nc) as tc:
        with (
            tc.tile_pool(name="const", bufs=1) as constp,
            tc.tile_pool(name="persist", bufs=1) as pers,
            tc.tile_pool(name="dram", bufs=1, space="DRAM") as dram,
            tc.tile_pool(name="stats", bufs=2) as statp,
            tc.tile_pool(name="vec", bufs=4) as vecp,
        ):
            ident = constp.tile([128, 128], F32, tag="ident")
            make_identity(nc, ident[:])
            identh = constp.tile([128, 128], F16, tag="identh")
            make_identity(nc, identh[:])

            # ---- load params into SBUF ----
            def load(name, shape, dt=F32, pool=constp):
                t = pool.tile(list(shape), dt, tag=name)
                nc.sync.dma_start(t[:], inp[name][:])
                return t

            Lt1 = load("Lt1", (P1_ROWS, N), F16)
            Rt1 = load("Rt1", (P1_ROWS, N), F16)
            W1T = load("W1T", (C1_IN, C1_OUT))
            W2T = load("W2T", (C1_OUT, C2_OUT))
            W3T = load("W3T", (C2_OUT, C3_OUT))
            Wg1T = load("Wg1T", (C3_OUT, G1_OUT), F16)
            Wg2T = load("Wg2T", (G1_OUT, G2_OUT), F16)
            W4Tp = load("W4Tp", (128, 8 * C4_OUT), F16)
            pv = {nm: load(nm, (64, 1)) for nm in
                  ("b1", "gm1", "bt1", "b2", "gm2", "bt2", "b3", "gm3", "bt3")}
            pv.update({nm: load(nm, (128, 1)) for nm in ("bg1", "gmg1", "btg1")})
            pv.update({nm: load(nm, (128, 8)) for nm in ("bg2", "gmg2", "btg2")})
            pv.update({nm: load(nm, (128, 4)) for nm in ("b4", "gm4", "bt4")})

            ones128 = constp.tile([128, 1], F32, tag="ones128")
            nc.vector.memset(ones128[:], 1.0)
            ones_rowh = constp.tile([2, 128], F16, tag="ones_rowh")
            nc.vector.memset(ones_rowh[:], 1.0)
            negones = constp.tile([128, 1], F32, tag="negones")
            nc.vector.memset(negones[:], -1.0)
            onesh = constp.tile([128, 1], F16, tag="onesh")
            nc.vector.memset(onesh[:], 1.0)
            # one shared gpsimd register for every dma_gather's num_idxs
            # (to_reg allocates a fresh register per call otherwise)
            nidx_reg = nc.gpsimd.to_reg(N)

            # persistent activations
            hT = pers.tile([12, N], F32, tag="hT")
            h4 = pers.tile([64, N], F32, tag="h4")
            h5 = pers.tile([128, N], F16, tag="h5")
            mT = pers.tile([64, N], F16, tag="mT")
            m2T = pers.tile([128, N], F16, tag="m2T")

            g1t_dt = F32 if USE_DMA_GATHER else F16
            ftbl = dram.tile([N, 64], g1t_dt, tag="ftbl")
            ft2bl = dram.tile([N, 128], F16, tag="ft2bl")

            # ---------------- BN helper ----------------
            coll_seq = [0]

            def allreduce_stats(stats):
                """AllReduce a (128, 16) f32 stats tile across all cores."""
                i = coll_seq[0]
                coll_seq[0] += 1
                in_b = dram.tile([128, 16], F32, tag=f"arin{i}")
                out_b = dram.tile([128, 16], F32, tag=f"arout{i}")
                nc.sync.dma_start(in_b[:], stats[:])
                nc.gpsimd.collective_compute(
                    "AllReduce", OP.add,
                    replica_groups=[list(range(N_CORES))],
                    ins=[in_b.opt()], outs=[out_b.opt()],
                )
                back = statp.tile([128, 16], F32, tag="arback")
                nc.sync.dma_start(back[:], out_b[:])
                return back

            def bn_vectors(gst, col_s, col_q, gm_ap, bt_ap, c):
                """From summed stats -> (a, sh) APs of shape (c,1)."""
                v = vecp.tile([128, 8], F32, tag="bnv")
                s = gst[0:c, col_s : col_s + 1]
                q = gst[0:c, col_q : col_q + 1]
                nc.vector.tensor_scalar_mul(v[0:c, 0:1], s, INV_M)            # mu
                nc.vector.tensor_scalar_mul(v[0:c, 1:2], q, INV_M)            # E[y^2]
                nc.vector.tensor_mul(v[0:c, 2:3], v[0:c, 0:1], v[0:c, 0:1])   # mu^2
                nc.vector.tensor_sub(v[0:c, 1:2], v[0:c, 1:2], v[0:c, 2:3])   # var
                nc.vector.tensor_scalar_add(v[0:c, 1:2], v[0:c, 1:2], BN_EPS)
                nc.scalar.sqrt(v[0:c, 2:3], v[0:c, 1:2])                      # std
                nc.vector.reciprocal(v[0:c, 3:4], v[0:c, 2:3])                # 1/std
                nc.vector.tensor_mul(v[0:c, 4:5], gm_ap, v[0:c, 3:4])         # a
                nc.vector.tensor_mul(v[0:c, 5:6], v[0:c, 0:1], v[0:c, 4:5])   # mu*a
                nc.vector.tensor_sub(v[0:c, 6:7], bt_ap, v[0:c, 5:6])         # sh
                return v

            def wrap_idx(idx, smp, dramp, first):
                """(128,16) u32 neighbor idx -> (128,128) i16 wrapped layout
                for dma_gather (rows 0..15 hold idx of linear pos s*128+p at
                [p%16, 8s+p//16]); via a DRAM round-trip shuffle.

                Rows 16..127 are never read by the gather ucode but the sim
                bounds-checks them; zero them once per ring slot (the shuffle
                DMA fully rewrites rows 0..15 on reuse)."""
                idx16 = smp.tile([128, 16], I16, tag="idx16")
                nc.vector.tensor_copy(idx16[:], idx[:])
                scratch = dramp.tile([128, 16], I16, tag="iscr")
                nc.sync.dma_start(scratch[:], idx16[:])
                xw = smp.tile([128, 128], I16, tag="xw")
                if first:
                    nc.gpsimd.memset(xw[:], 0)
                dv = xw[0:16, :].rearrange("a (s b) -> a s b", b=8)
                srcv = scratch[:].rearrange("(b a) s -> a s b", b=8)
                nc.sync.dma_start(dv, srcv)
                return xw

            # ---------------- phase 1: knn on xyz -> covariance features ----
            # Gather-free: after top-16 VALUES (max8/mr8/max8), the selection
            # mask M[p,j] = (ut[p,j] >= v16[p]) is built in one DVE pass; the
            # neighbor sums [S(3), SS(9)] come from PE matmuls M^T-chunk @ P12
            # (hi/lo fp16 product table), so no indirect DMAs and no
            # FIND_INDEX8 passes at all. cov = SS - S S^T/16.
            P12sb = constp.tile([128, 16 * 24], F16, tag="P12sb")
            nc.sync.dma_start(
                P12sb[:].rearrange("p (c f) -> p c f", f=24),
                inp["P12"][:].rearrange("(c p) f -> p c f", p=128))
            with (
                tc.tile_pool(name="k1_psum", bufs=1, space="PSUM") as up,
                tc.tile_pool(name="k1_tp", bufs=2, space="PSUM") as tpp,
                tc.tile_pool(name="k1_cb", bufs=2, space="PSUM") as cbp,
                tc.tile_pool(name="k1_ut", bufs=2) as utp,
                tc.tile_pool(name="k1_scr", bufs=2) as scrp,
                tc.tile_pool(name="k1_small", bufs=3) as smp,
                tc.tile_pool(name="k1_mt", bufs=2) as mtp,
            ):
                def p1_topk(i):
                    """distmat + top-16 values + selection mask (DVE)."""
                    pu = up.tile([128, N], F32, tag="u")
                    for j in range(4):
                        nc.tensor.matmul(pu[:, ts(j, 512)], Lt1[:, ts(i, 128)],
                                         Rt1[:, ts(j, 512)])
                    ut = utp.tile([128, N], F32, tag="ut")
                    nc.scalar.activation(ut[:], pu[:], AF.Identity)
                    m8 = smp.tile([128, 16], F32, tag="m8")
                    scr = scrp.tile([128, N], F32, tag="scr")
                    nc.vector.max(m8[:, 0:8], ut[:])
                    nc.vector.match_replace(scr[:], m8[:, 0:8], ut[:], NEG)
                    nc.vector.max(m8[:, 8:16], scr[:])
                    # selection mask (0/1 fp16) via a second match_replace
                    # (first-occurrence replace == reference lowest-index
                    # tie-break): scr2 has all top-16 = NEG, so ut - scr2 is
                    # 0 for non-selected and ~1e30 for selected; min(.,1)
                    # gives exact 0/1.  (tensor_scalar is_ge miscompares on
                    # HW DVE despite simulating correctly.)
                    scr2 = scrp.tile([128, N], F32, tag="scr2")
                    nc.vector.match_replace(scr2[:], m8[:, 8:16], scr[:], NEG)
                    dif = scrp.tile([128, N], F32, tag="dif")
                    nc.vector.tensor_sub(dif[:], ut[:], scr2[:])
                    Mm = scrp.tile([128, N], F16, tag="Mm")
                    nc.vector.tensor_scalar_min(Mm[:], dif[:], 1.0)
                    xt = smp.tile([128, 4], F32, tag="xt")
                    nc.sync.dma_start(xt[:], inp["xpad"][ts(i, 128), 0:4])
                    return Mm, xt

                def p1_maskmm(i, Mm):
                    """[S,SS] += (M^T chunk) @ P12 on PE (lags topk by one
                    block so the PE queue never stalls the next distmat)."""
                    pcb = cbp.tile([128, 24], F32, tag="pcb")
                    MT = mtp.tile([128, N], F16, tag="MT")
                    for c in range(NB):
                        mtps = tpp.tile([128, 128], F16, tag="mtps")
                        nc.tensor.transpose(mtps[:], Mm[:, ts(c, 128)],
                                            identh[:])
                        nc.scalar.activation(MT[:, ts(c, 128)], mtps[:],
                                             AF.Identity)
                        nc.tensor.matmul(pcb[:], MT[:, ts(c, 128)],
                                         P12sb[:, 24 * c : 24 * c + 24],
                                         start=(c == 0), stop=(c == NB - 1))
                    return pcb

                def p1_cov(i, pcb, xt):
                    """covariance features for block i from [S,SS] sums."""
                    s24 = smp.tile([128, 24], F32, tag="s24")
                    nc.scalar.activation(s24[:], pcb[:], AF.Identity)
                    cb = smp.tile([128, 12], F32, tag="cb")
                    nc.vector.tensor_copy(cb[:, 0:3], xt[:, 0:3])
                    # hi+lo halves -> S (cols 0:3), SS (cols 3:12)
                    nc.vector.tensor_add(s24[:, 0:12], s24[:, 0:12],
                                         s24[:, 12:24])
                    nc.vector.tensor_copy(cb[:, 3:12], s24[:, 3:12])
                    # subtract S_c*S_d/16
                    m3 = smp.tile([128, 3], F32, tag="m3")
                    nc.vector.tensor_scalar_mul(m3[:], s24[:, 0:3], 0.25)
                    p16 = smp.tile([128, 9], F32, tag="p16")
                    for c in range(3):
                        nc.vector.tensor_scalar_mul(
                            p16[:, 3 * c : 3 * c + 3], m3[:], m3[:, c : c + 1])
                    nc.vector.tensor_sub(cb[:, 3:12], cb[:, 3:12], p16[:])
                    # transpose (128, 12) -> (12, 128) into hT, via fp16 so
                    # the transpose shares the mtps PSUM slots (bank budget)
                    cb16 = smp.tile([128, 12], F16, tag="cb16")
                    nc.scalar.activation(cb16[:], cb[:], AF.Identity)
                    ptp = tpp.tile([128, 128], F16, tag="mtps")
                    nc.tensor.transpose(ptp[0:12, :], cb16[:], identh[:])
                    nc.scalar.activation(hT[0:12, ts(i, 128)], ptp[0:12, :],
                                         AF.Identity)

                # 2-deep software pipeline: topk(i) | maskmm(i-1) | cov(i-2)
                mask_pend = None
                cov_pend = None
                for i in range(NB):
                    mk = p1_topk(i)
                    if cov_pend is not None:
                        p1_cov(*cov_pend)
                        cov_pend = None
                    if mask_pend is not None:
                        im, Mm_, xt_ = mask_pend
                        cov_pend = (im, p1_maskmm(im, Mm_), xt_)
                    mask_pend = (i, *mk)
                if cov_pend is not None:
                    p1_cov(*cov_pend)
                im, Mm_, xt_ = mask_pend
                p1_cov(im, p1_maskmm(im, Mm_), xt_)
            # ---------------- phase 2: conv1..conv3 ----------------
            scr2 = pers.tile([128, N], F32, tag="sqscr")

            def conv_bn_small(rhs_ap, WT, cin, cout, b, gm, bt, h_out,
                              gst_pre=None):
                with tc.tile_pool(name="conv_psum", bufs=1, space="PSUM") as cp:
                    py = cp.tile([cout, N], F32, tag="y")
                    for j in range(4):
                        nc.tensor.matmul(py[:, ts(j, 512)], WT[:],
                                         rhs_ap[:, ts(j, 512)])
                    hpre = h_out[0:cout, :]
                    if gst_pre is None:
                        stats = statp.tile([128, 16], F32, tag="st")
                        nc.vector.memset(stats[:], 0.0)
                        nc.scalar.activation(hpre, py[:], AF.Identity,
                                             bias=b[0:cout, 0:1],
                                             accum_out=stats[0:cout, 0:1])
                        nc.scalar.activation(scr2[0:cout, :], hpre, AF.Square,
                                             accum_out=stats[0:cout, 1:2])
                        gst = allreduce_stats(stats)
                    else:
                        # stats precomputed analytically; AR already in
                        # flight and overlaps these matmuls/copies
                        nc.scalar.activation(hpre, py[:], AF.Identity,
                                             bias=b[0:cout, 0:1])
                        gst = gst_pre
                    v = bn_vectors(gst, 0, 1, gm[0:cout, 0:1], bt[0:cout, 0:1], cout)
                    nc.scalar.activation(hpre, hpre, AF.Relu,
                                         scale=v[0:cout, 4:5], bias=v[0:cout, 6:7])

            def analytic_stats(momt, W32, b, cin, cout, tag):
                """BN stats from moment sums. momt: PSUM AP [cin, cin+1] =
                [M | s] with M = sum_n f f^T, s = sum_n f (over this core's
                n=N points). stats col0 = W s + N b, col1 = diag(W M W^T) +
                2 b (W s) + N b^2.  Returns the in-flight AR'd stats tile."""
                stats = statp.tile([128, 16], F32, tag=f"ast{tag}")
                nc.vector.memset(stats[:], 0.0)
                with (
                    tc.tile_pool(name=f"as_ps{tag}", bufs=1, space="PSUM") as aps,
                    tc.tile_pool(name=f"as_sb{tag}", bufs=1) as asb,
                ):
                    Ms = asb.tile([cin, cin + 1], F32, tag="Ms")
                    nc.scalar.activation(Ms[:], momt, AF.Identity)
                    pP = aps.tile([cin, cout], F32, tag="pP")
                    nc.tensor.matmul(pP[:], Ms[:, 0:cin], W32[:])
                    Q32 = asb.tile([cin, cout], F32, tag="Q32")
                    nc.vector.tensor_mul(Q32[:], pP[:], W32[:])
                    pq = aps.tile([cout, 2], F32, tag="pq")
                    nc.tensor.matmul(pq[:, 0:1], Q32[:], ones128[0:cin, :])
                    nc.tensor.matmul(pq[:, 1:2], W32[:], Ms[:, cin : cin + 1])
                    qs = asb.tile([cout, 2], F32, tag="qs")
                    nc.scalar.activation(qs[:], pq[:], AF.Identity)
                    t4 = vecp.tile([128, 1], F32, tag=f"ast4{tag}")
                    bc = b[0:cout, 0:1]
                    nc.vector.tensor_scalar_mul(t4[0:cout, :], bc, float(N))
                    nc.vector.tensor_add(stats[0:cout, 0:1], qs[:, 1:2],
                                         t4[0:cout, :])
                    nc.vector.tensor_mul(t4[0:cout, :], bc, qs[:, 1:2])
                    nc.vector.tensor_scalar_mul(t4[0:cout, :], t4[0:cout, :], 2.0)
                    nc.vector.tensor_add(stats[0:cout, 1:2], qs[:, 0:1],
                                         t4[0:cout, :])
                    nc.vector.tensor_mul(t4[0:cout, :], bc, bc)
                    nc.vector.tensor_scalar_mul(t4[0:cout, :], t4[0:cout, :],
                                                float(N))
                    nc.vector.tensor_add(stats[0:cout, 1:2],
                                         stats[0:cout, 1:2], t4[0:cout, :])
                return allreduce_stats(stats)

            with tc.tile_pool(name="hpre_pool", bufs=2) as scrp2:
                conv_bn_small(hT[:], W1T, C1_IN, 64, pv["b1"], pv["gm1"],
                              pv["bt1"], h4)
                conv_bn_small(h4[:], W2T, 64, 64, pv["b2"], pv["gm2"], pv["bt2"], h4)
                conv_bn_small(h4[:], W3T, 64, 64, pv["b3"], pv["gm3"], pv["bt3"], h4)

                # feature table (N, 64) f32 for g1 dma_gather (256B rows)
                with tc.tile_pool(name="ft_psum", bufs=2, space="PSUM") as ftp:
                    for i in range(NB):
                        ptp = ftp.tile([128, 64], F32, tag="ftT")
                        nc.tensor.transpose(ptp[:], h4[:, ts(i, 128)],
                                            ident[0:64, 0:64])
                        ft = scrp2.tile([128, 64], g1t_dt, tag="fts")
                        nc.scalar.activation(ft[:], ptp[:], AF.Identity)
                        nc.sync.dma_start(ftbl[ts(i, 128), :], ft[:])

            # ---------------- graph layer helper ----------------
            def graph_knn(feat, cdim, ftable, pooled_T, g_dt, mom=None):
                """kNN in feature space + gather + max-pool; writes pooled^T
                (cdim, N) fp16 into pooled_T. feat: (cdim, N) f32 or fp16.

                Distance matmuls run on fp16 inputs with f32 PSUM accum; the
                -|f|^2 column term rides as fp16 hi/lo contraction rows (fold
                into one matmul when cdim+2 <= 128, else a separate 2-row
                matmul); the -|f|^2 row term is a f32 per-partition bias at the
                PSUM->SBUF copy, recentering ut to -d. Top-k compare is f32."""
                fold = cdim + 2 <= 128
                with tc.tile_pool(name="gk_sb", bufs=1) as sb:
                    with tc.tile_pool(name="gk_prep", bufs=1, space="PSUM") as pp:
                        # aa[n] = sum_c feat[c,n]^2  (via ones-vector matmul)
                        nc.scalar.activation(scr2[0:cdim, :], feat[:], AF.Square)
                        pa = pp.tile([1, N], F32, tag="aa")
                        for j in range(4):
                            nc.tensor.matmul(pa[:, ts(j, 512)], ones128[0:cdim, :],
                                             scr2[0:cdim, ts(j, 512)])
                        # (no per-partition -|f_p|^2 recentering: a
                        # per-partition constant cannot change that row's
                        # top-k selection, and compare values are f32)
                        # hi/lo fp16 split of -aa for the matmul free-axis term
                        hi_h = sb.tile([1, N], F16, tag="hi_h")
                        nc.scalar.activation(hi_h[:], pa[:], AF.Identity,
                                             scale=-1.0)
                        hi_f = sb.tile([1, N], F32, tag="hi_f")
                        nc.scalar.activation(hi_f[:], hi_h[:], AF.Identity)
                        naa_f = sb.tile([1, N], F32, tag="naa_f")
                        nc.scalar.activation(naa_f[:], pa[:], AF.Identity,
                                             scale=-1.0)
                        lo_h = sb.tile([1, N], F16, tag="lo_h")
                        nc.vector.tensor_sub(lo_h[:], naa_f[:], hi_f[:])
                        if fold:
                            Lt = sb.tile([cdim + 2, N], F16, tag="lt")
                            Rt = sb.tile([cdim + 2, N], F16, tag="rt")
                            nc.scalar.activation(Lt[0:cdim, :], feat[:],
                                                 AF.Identity, scale=2.0)
                            nc.vector.memset(Lt[cdim : cdim + 2, :], 1.0)
                            nc.scalar.activation(Rt[0:cdim, :], feat[:], AF.Identity)
                            nc.sync.dma_start(Rt[cdim : cdim + 1, :], hi_h[:])
                            nc.sync.dma_start(Rt[cdim + 1 : cdim + 2, :], lo_h[:])
                        else:
                            Lt = sb.tile([cdim, N], F16, tag="lt")
                            nc.scalar.activation(Lt[:], feat[:], AF.Identity,
                                                 scale=2.0)
                            aarows = sb.tile([2, N], F16, tag="aarows")
                            nc.sync.dma_start(aarows[0:1, :], hi_h[:])
                            nc.sync.dma_start(aarows[1:2, :], lo_h[:])
                            feat_h = feat

                    with (
                        tc.tile_pool(name="gk_psum", bufs=1, space="PSUM") as up,
                        tc.tile_pool(name="gk_tp", bufs=2, space="PSUM") as tpp,
                        tc.tile_pool(name="gk_ut", bufs=2) as utp,
                        tc.tile_pool(name="gk_scr", bufs=2) as scrp,
                        tc.tile_pool(name="gk_small", bufs=3) as smp,
                        tc.tile_pool(name="gk_g", bufs=4) as gp,
                        tc.tile_pool(name="gk_dram", bufs=3, space="DRAM") as dramp,
                    ):
                        def gk_stage_a(i):
                            """top-k + gather launch for block i."""
                            pu = up.tile([128, N], F32, tag="u")
                            for j in range(4):
                                if fold:
                                    nc.tensor.matmul(pu[:, ts(j, 512)],
                                                     Lt[:, ts(i, 128)],
                                                     Rt[:, ts(j, 512)])
                                else:
                                    nc.tensor.matmul(pu[:, ts(j, 512)],
                                                     Lt[:, ts(i, 128)],
                                                     feat_h[:, ts(j, 512)],
                                                     start=True, stop=False)
                                    nc.tensor.matmul(pu[:, ts(j, 512)],
                                                     ones_rowh[:, 0:128],
                                                     aarows[:, ts(j, 512)],
                                                     start=False, stop=True)
                            ut = utp.tile([128, N], F32, tag="ut")
                            nc.scalar.activation(ut[:], pu[:], AF.Identity)
                            m8 = smp.tile([128, 16], F32, tag="m8")
                            scr = scrp.tile([128, N], F32, tag="scr")
                            nc.vector.max(m8[:, 0:8], ut[:])
                            nc.vector.match_replace(scr[:], m8[:, 0:8], ut[:], NEG)
                            nc.vector.max(m8[:, 8:16], scr[:])
                            idx = smp.tile([128, 16], U32, tag="idx")
                            nc.vector.max_index(idx[:, 0:8], m8[:, 0:8], ut[:])
                            nc.vector.max_index(idx[:, 8:16], m8[:, 8:16], ut[:])

                            g = gp.tile([128, 16 * cdim], g_dt, tag="g")
                            if USE_DMA_GATHER:
                                xw = wrap_idx(idx, smp, dramp, i < 3)
                                nc.gpsimd.dma_gather(
                                    g[:].rearrange("p (s c) -> p s c", c=cdim),
                                    ftable[:], xw[:], num_idxs=N,
                                    num_idxs_reg=nidx_reg, elem_size=cdim)
                            else:
                                for sx in range(16):
                                    nc.gpsimd.indirect_dma_start(
                                        g[:, cdim * sx : cdim * (sx + 1)], None,
                                        ftable[:],
                                        IndirectOffsetOnAxis(
                                            ap=idx[:, sx : sx + 1], axis=0))
                            return g

                        def gk_stage_b(i, g):
                            """max-pool + transpose for block i."""
                            w = 8 * cdim
                            nc.vector.tensor_tensor(g[:, 0:w], g[:, 0:w],
                                                    g[:, w : 2 * w], op=OP.max)
                            w //= 2
                            nc.vector.tensor_tensor(g[:, 0:w], g[:, 0:w],
                                                    g[:, w : 2 * w], op=OP.max)
                            w //= 2
                            nc.vector.tensor_tensor(g[:, 0:w], g[:, 0:w],
                                                    g[:, w : 2 * w], op=OP.max)
                            w //= 2
                            nc.vector.tensor_tensor(g[:, 0:w], g[:, 0:w],
                                                    g[:, w : 2 * w], op=OP.max)
                            if mom is not None:
                                # fused moment sums [M | s] += g^T [g | 1] on
                                # PE -- ONE accumulation chain (two
                                # interleaved open chains in one PSUM bank
                                # corrupt each other on HW)
                                nc.vector.memset(g[:, cdim : cdim + 1], 1.0)
                                nc.tensor.matmul(mom, g[:, 0:cdim],
                                                 g[:, 0 : cdim + 1],
                                                 start=(i == 0),
                                                 stop=(i == NB - 1))
                            if g_dt == F16:
                                ptp = tpp.tile([cdim, 128], F16, tag="plT")
                                nc.tensor.transpose(ptp[:], g[:, 0:cdim],
                                                    identh[:])
                            else:
                                ptp = tpp.tile([cdim, 128], F32, tag="plTf")
                                nc.tensor.transpose(ptp[:], g[:, 0:cdim],
                                                    ident[:])
                            nc.scalar.activation(pooled_T[:, ts(i, 128)], ptp[:],
                                                 AF.Identity)

                        # software-pipelined: B lags A by two iterations so
                        # the DVE stream never waits on a gather's completion
                        pend = []
                        for i in range(NB):
                            pend.append((i, gk_stage_a(i)))
                            if len(pend) > 2:
                                ib, g_ = pend.pop(0)
                                gk_stage_b(ib, g_)
                        for ib, g_ in pend:
                            gk_stage_b(ib, g_)

            # ---------------- phase 3: graph layer 1 ----------------
            Wg1T32 = constp.tile([64, 128], F32, tag="Wg1T32")
            nc.scalar.activation(Wg1T32[:], Wg1T[:], AF.Identity)
            with tc.tile_pool(name="g1mom", bufs=1, space="PSUM") as momg1:
                momtg1 = momg1.tile([64, 65], F32, tag="momtg1")
                graph_knn(h4, 64, ftbl, mT, g1t_dt,
                          mom=momtg1[0:64, 0:65])
                gstg1 = analytic_stats(momtg1[0:64, :], Wg1T32, pv["bg1"],
                                       64, 128, "g1")
            with tc.tile_pool(name="hpre_pool2", bufs=2) as scrp2b:
                conv_bn_small(mT[:], Wg1T, 64, 128, pv["bg1"], pv["gmg1"],
                              pv["btg1"], h5, gst_pre=gstg1)
                with tc.tile_pool(name="ft2_psum", bufs=2, space="PSUM") as ftp:
                    for i in range(NB):
                        ptp = ftp.tile([128, 128], F16, tag="ft2T")
                        nc.tensor.transpose(ptp[:], h5[:, ts(i, 128)], identh[:])
                        ft = scrp2b.tile([128, 128], F16, tag="ft2s")
                        nc.scalar.activation(ft[:], ptp[:], AF.Identity)
                        nc.sync.dma_start(ft2bl[ts(i, 128), :], ft[:])

            # ---------------- phase 4: graph layer 2 + convg2 ----------------
            # BN stats for convg2 come analytically from the pooled features'
            # moment sums, accumulated on the (idle) PE during the knn phase:
            #   sum_n y      = W s + n b
            #   sum_n y^2    = diag(W M W^T) + 2 b (W s) + n b^2
            # so the stats AllReduce fires right at phase end and overlaps the
            # convg2 matmuls; the 8 per-mblk SQUARE passes disappear.
            stats = statp.tile([128, 16], F32, tag="stg2")
            with tc.tile_pool(name="g2mom", bufs=1, space="PSUM") as momp:
                mom_Ms = momp.tile([128, 129], F32, tag="Ms2")
                graph_knn(h5, 128, ft2bl, m2T, F16,
                          mom=mom_Ms[0:128, 0:129])

                with (
                    tc.tile_pool(name="g2st_psum", bufs=1, space="PSUM") as sp2,
                    tc.tile_pool(name="g2st_sb", bufs=1) as sb2,
                ):
                    # hi/lo fp16 split of M (values up to ~n, fp16 alone
                    # would quantize at ~5e-4; hi/lo keeps ~f32)
                    M32 = sb2.tile([128, 128], F32, tag="M32")
                    nc.scalar.activation(M32[:], mom_Ms[:, 0:128], AF.Identity)
                    Mh = sb2.tile([128, 128], F16, tag="Mh")
                    nc.scalar.activation(Mh[:], M32[:], AF.Identity)
                    Mhf = sb2.tile([128, 128], F32, tag="Mhf")
                    nc.scalar.activation(Mhf[:], Mh[:], AF.Identity)
                    Ml = sb2.tile([128, 128], F16, tag="Ml")
                    nc.vector.tensor_sub(Ml[:], M32[:], Mhf[:])
                    sh = sb2.tile([128, 1], F16, tag="sh")
                    nc.scalar.activation(sh[:], mom_Ms[:, 128:129], AF.Identity)
                    # f32 copy of Wg2T for the Hadamard step
                    W32 = sb2.tile([128, 8 * 128], F32, tag="W32")
                    nc.scalar.activation(W32[:], Wg2T[:], AF.Identity)

                    psy = sp2.tile([128, 8], F32, tag="psy")
                    pq = sp2.tile([128, 8], F32, tag="pq")
                    for mb in range(8):
                        Wmb = Wg2T[:, ts(mb, 128)]
                        pP = sp2.tile([128, 128], F32, tag="pP")
                        nc.tensor.matmul(pP[:], Mh[:], Wmb,
                                         start=True, stop=False)
                        nc.tensor.matmul(pP[:], Ml[:], Wmb,
                                         start=False, stop=True)
                        Q32 = sb2.tile([128, 128], F32, tag="Q32")
                        nc.vector.tensor_mul(Q32[:], pP[:],
                                             W32[:, ts(mb, 128)])
                        nc.tensor.matmul(pq[:, mb : mb + 1], Q32[:],
                                         ones128[:])
                        nc.tensor.matmul(psy[:, mb : mb + 1], Wmb, sh[:])
                    syv = sb2.tile([128, 8], F32, tag="syv")
                    nc.scalar.activation(syv[:], psy[:], AF.Identity)
                    qv = sb2.tile([128, 8], F32, tag="qv")
                    nc.scalar.activation(qv[:], pq[:], AF.Identity)
                    # stats[:, 0:8] = syv + n*b ; stats[:, 8:16] =
                    #   qv + 2 b syv + n b^2   (n = N points per core)
                    bg2 = pv["bg2"]
                    t8 = sb2.tile([128, 8], F32, tag="t8")
                    nc.vector.tensor_scalar_mul(t8[:], bg2[:], float(N))
                    nc.vector.tensor_add(stats[:, 0:8], syv[:], t8[:])
                    nc.vector.tensor_mul(t8[:], bg2[:], syv[:])
                    nc.vector.tensor_scalar_mul(t8[:], t8[:], 2.0)
                    nc.vector.tensor_add(stats[:, 8:16], qv[:], t8[:])
                    nc.vector.tensor_mul(t8[:], bg2[:], bg2[:])
                    nc.vector.tensor_scalar_mul(t8[:], t8[:], float(N))
                    nc.vector.tensor_add(stats[:, 8:16], stats[:, 8:16],
                                         t8[:])

            gst = allreduce_stats(stats)

            # convg2: (1024, 128) @ (128, N) -- matmuls overlap the AllReduce
            latep_cm = tc.tile_pool(name="late", bufs=1)
            latep = latep_cm.__enter__()
            h6 = [latep.tile([128, N], F16, tag=f"h6_{j}", name=f"h6_{j}")
                  for j in range(8)]
            with tc.tile_pool(name="g2conv_psum", bufs=2, space="PSUM") as cp:
                for mblk in range(8):
                    py = cp.tile([128, N], F32, tag="y")
                    for j in range(4):
                        nc.tensor.matmul(py[:, ts(j, 512)],
                                         Wg2T[:, ts(mblk, 128)],
                                         m2T[:, ts(j, 512)])
                    nc.scalar.activation(h6[mblk][:], py[:], AF.Identity,
                                         bias=pv["bg2"][:, mblk : mblk + 1])
                for mblk in range(8):
                    v = bn_vectors(gst, mblk, 8 + mblk,
                                   pv["gmg2"][:, mblk : mblk + 1],
                                   pv["btg2"][:, mblk : mblk + 1], 128)
                    nc.scalar.activation(h6[mblk][:], h6[mblk][:], AF.Relu,
                                         scale=v[:, 4:5], bias=v[:, 6:7])

            # ---------------- phase 5: conv4 + BN4 + global max ----------------
            with (
                tc.tile_pool(name="c4_psum", bufs=2, space="PSUM") as cp,
                tc.tile_pool(name="c4_sb", bufs=2) as hp,
            ):
                stats = statp.tile([128, 16], F32, tag="st4")
                nc.vector.memset(stats[:], 0.0)
                maxc = pers.tile([128, 4], F32, tag="maxc")
                for mblk in range(4):
                    py = cp.tile([128, N], F32, tag="y")
                    for j in range(4):
                        for k in range(8):
                            nc.tensor.matmul(
                                py[:, ts(j, 512)],
                                W4Tp[:, 512 * k + 128 * mblk : 512 * k + 128 * mblk + 128],
                                h6[k][:, ts(j, 512)],
                                start=(k == 0), stop=(k == 7))
                    y4 = hp.tile([128, N], F32, tag="y4")
                    nc.scalar.activation(y4[:], py[:], AF.Identity,
                                         bias=pv["b4"][:, mblk : mblk + 1],
                                         accum_out=stats[:, mblk : mblk + 1])
                    nc.scalar.activation(scr2[:], y4[:], AF.Square,
                                         accum_out=stats[:, 8 + mblk : 9 + mblk])
                    nc.vector.reduce_max(maxc[:, mblk : mblk + 1], y4[:], axis=AX.X)
                gst = allreduce_stats(stats)
                out4 = pers.tile([128, 4], F32, tag="out4")
                for mblk in range(4):
                    v = bn_vectors(gst, mblk, 8 + mblk,
                                   pv["gm4"][:, mblk : mblk + 1],
                                   pv["bt4"][:, mblk : mblk + 1], 128)
                    # out = (max - mu) * a + bt  (valid since gm>0)
                    nc.vector.tensor_sub(out4[:, mblk : mblk + 1],
                                         maxc[:, mblk : mblk + 1], v[:, 0:1])
                    nc.vector.tensor_mul(out4[:, mblk : mblk + 1],
                                         out4[:, mblk : mblk + 1], v[:, 4:5])
                    nc.vector.tensor_add(out4[:, mblk : mblk + 1],
                                         out4[:, mblk : mblk + 1],
                                         pv["bt4"][:, mblk : mblk + 1])
            with (
                tc.tile_pool(name="fin_psum", bufs=1, space="PSUM") as fp,
                tc.tile_pool(name="fin_sb", bufs=1) as fsb,
            ):
                ptp = fp.tile([4, 128], F32, tag="outT")
                nc.tensor.transpose(ptp[:], out4[:], ident[:])
                outs = fsb.tile([4, 128], F32, tag="outs")
                nc.scalar.activation(outs[:], ptp[:], AF.Identity)
                nc.sync.dma_start(out_t[:], outs[:])
            latep_cm.__exit__(None, None, None)

    # auto-insert gpsimd library reloads (dma_gather lives in the mlp
    # library) and generate ISA bytes for the inserted MODIFY_POOL_CONFIG
    # instructions -- walrus rejects empty .instr with "ISA wrong length"
    inst_type_to_lib_mask = {}
    for lib in all_libraries:
        for it in lib.instructions:
            inst_type_to_lib_mask[it] = (
                inst_type_to_lib_mask.get(it, 0) | (1 << lib.index))
    bass_rust.insert_library_loads(
        nc, inst_type_to_lib_mask, len(all_libraries), standard.index)
    mybir.codegen_inst_isa_subclasses(nc)
    split_drain_waits(nc)
    return nc


_PROGRAM = None


def _get_program():
    global _PROGRAM
    if _PROGRAM is None:
        _PROGRAM = build_program()
    return _PROGRAM


def make_in_maps(x, weights):
    """x: (B, N, 3); weights: dict of the reference param arrays."""
    f16 = np.float16
    shared = {}
    shared["W1T"] = np.ascontiguousarray(weights["W1"].T)
    shared["W2T"] = np.ascontiguousarray(weights["W2"].T)
    shared["W3T"] = np.ascontiguousarray(weights["W3"].T)
    shared["Wg1T"] = np.ascontiguousarray(weights["Wg1"].T).astype(f16)
    shared["Wg2T"] = np.ascontiguousarray(weights["Wg2"].T).astype(f16)
    W4 = weights["W4"]
    chunks = [np.ascontiguousarray(W4[:, 128 * j : 128 * (j + 1)].T) for j in range(8)]
    shared["W4Tp"] = np.ascontiguousarray(np.concatenate(chunks, axis=1)).astype(f16)
    for nm in ("b1", "gm1", "bt1", "b2", "gm2", "bt2", "b3", "gm3", "bt3"):
        shared[nm] = np.ascontiguousarray(weights[nm].reshape(-1, 1))
    for nm in ("bg1", "gmg1", "btg1"):
        shared[nm] = np.ascontiguousarray(weights[nm].reshape(-1, 1))
    for nm in ("bg2", "gmg2", "btg2"):
        shared[nm] = np.ascontiguousarray(weights[nm].reshape(8, 128).T)
    for nm in ("b4", "gm4", "bt4"):
        shared[nm] = np.ascontiguousarray(weights[nm].reshape(4, 128).T)

    in_maps = []
    for c in range(B):
        xc = np.asarray(x[c], dtype=np.float32)       # (N, 3)
        xT = np.ascontiguousarray(xc.T)               # (3, N)
        aa = (xc * xc).sum(axis=1).astype(np.float32)  # (N,)
        m = dict(shared)
        # phase-1 distmat via exact fp16 hi/lo split:
        # 2x.x' = 2xh.x'h + 2xh.x'l + 2xl.x'h (+O(2^-22));  aa = aah + aal
        xh = xT.astype(f16)
        xl = (xT - xh.astype(np.float32)).astype(f16)
        aah = aa.astype(f16)
        aal = (aa - aah.astype(np.float32)).astype(f16)
        m["Lt1"] = np.ascontiguousarray(np.concatenate(
            [2.0 * xh, 2.0 * xh, 2.0 * xl,
             np.ones((2, N), f16)], axis=0).astype(f16))
        m["Rt1"] = np.ascontiguousarray(np.concatenate(
            [xh, xl, xh, -aah[None, :], -aal[None, :]], axis=0).astype(f16))
        m["xpad"] = np.ascontiguousarray(
            np.concatenate([xc, np.zeros((N, 1), np.float32)], axis=1))
        # product table for the mask-matmul covariance path: per point
        # [x(3), x_c*x_d(9)] split hi/lo fp16 -> (N, 24)
        prods = (xc[:, :, None] * xc[:, None, :]).reshape(N, 9)
        P = np.concatenate([xc, prods], axis=1).astype(np.float32)  # (N, 12)
        Ph = P.astype(f16)
        Pl = (P - Ph.astype(np.float32)).astype(f16)
        m["P12"] = np.ascontiguousarray(np.concatenate([Ph, Pl], axis=1))
        in_maps.append(m)
    return in_maps


def kernel(**inputs):
    x = np.asarray(inputs["x"], dtype=np.float32)
    weights = {k: np.asarray(v, dtype=np.float32)
               for k, v in inputs.items() if k != "x"}
    nc = _get_program()
    in_maps = make_in_maps(x, weights)
    res = run_bass_kernel_spmd(nc, in_maps, core_ids=list(range(N_CORES)),
                               trace=False)
    out = np.stack([res.results[c]["out"].reshape(512) for c in range(B)])
    return out.astype(np.float32)


if __name__ == "__main__":
    nc = build_program()
    print("program built ok")

